# revision 25
# baseline (speedup 1.0000x reference)
"""EfficientAttention (linear attention) Trainium2 Bass kernel.

Computes, per batch b:
    q_n = softmax(q[b], axis=-1)        # over feature dim D=64
    k_n = softmax(k[b], axis=-1)
    ctx = k_n^T @ v[b]                  # [D, D]
    out[b] = q_n @ ctx                  # [N, D]

Sharding: batch dim (32) split across 8 cores, 4 batches per core.

Design notes (per core; memory-bound: 48 MB reads + 8 MB f16 writes at
~358 GB/s HBM-per-NC is the floor):
- k, v loaded f32 via HWDGE (sync queue); q via SWDGE (gpsimd queue)
  with in-flight f32->f16 cast, so q has its own DMA queue and arrives
  early while raw-q transposes become 1-pass fp16 with fast WL.
- kv pass: ctx = (ek/ks)^T v == ek^T (v/ks) -- ACT writes exp(k)
  directly as f16, DVE row-sums + reciprocal, and the softmax scale is
  fused into the v f32->f16 cast (one DVE broadcast multiply). No
  gpsimd in the chain.
- Output stored f16 (halves store traffic; host upcasts; 2e-2 budget).
- Q pass is split in two stages to keep every in-order engine stream
  consistent with DMA arrival order:
    qA (no ctx needed): load q, PE-transpose f16 pairs -> PSUM f16
       (full-bank tiles), ACT exp evict -> staged eqT tiles in SBUF.
    qB (needs ctxa): one matmul per tile pair into a single 2-bank
       PSUM tile (slot stride 1 KB so no 130-col write crosses a
       bank), ONE DVE reciprocal + ONE DVE normalize mul per chunk,
       deferred store trigger.
- ctx epilogue: block-diagonal stacked ctxa [128, 130] fp16 with ones
  columns so the q matmul also produces exp(q) row sums (cols 64/129).
- Schedule: slot s of batch b emits kv(b), qA(b), qB(b-1). qA(3) is
  reserved for the final group so its 4 MB of q loads (plus batch 3
  stores) keep the DMA busy while qB(3) drains; the exposed tail is
  one chunk chain, not a batch.
- Store triggers (ACT/HWDGE) are deferred one block so the ACT stream
  never stalls waiting on the DVE normalize.
"""

import numpy as np

import concourse.bass as bass
import concourse.mybir as mybir
import concourse.tile as tile
from concourse import bacc
from concourse.bass_utils import run_bass_kernel_spmd

B, N, D = 32, 16384, 64
NCORES = 8
BPC = B // NCORES  # batches per core
LOAD = 4096  # rows per DMA (1 MB fp32)
LT = LOAD // 128  # row-tile slots per load (32)
NBLK = N // LOAD  # load blocks per batch (4)
NCHUNK = LT // 8  # 1024-row compute chunks per block (4)
F32 = mybir.dt.float32
F16 = mybir.dt.float16
EXP = mybir.ActivationFunctionType.Exp


def build_bass():
    nc = bacc.Bacc("TRN2", target_bir_lowering=False, debug=False)
    q = nc.dram_tensor("q", [BPC, N, D], F32, kind="ExternalInput").ap()
    k = nc.dram_tensor("k", [BPC, N, D], F32, kind="ExternalInput").ap()
    v = nc.dram_tensor("v", [BPC, N, D], F32, kind="ExternalInput").ap()
    o = nc.dram_tensor("o", [BPC, N, D], F16, kind="ExternalOutput").ap()

    def blk(t, b, n0):
        return t[b, n0 : n0 + LOAD, :].rearrange("(p t) d -> p t d", p=128)

    with tile.TileContext(nc) as tc:
        with (
            tc.tile_pool(name="consts", bufs=1) as consts,
            tc.tile_pool(name="io", bufs=2) as io,
            tc.tile_pool(name="work", bufs=3) as work,
            tc.tile_pool(name="ctxp", bufs=3) as ctxp,
            tc.tile_pool(name="ps_t", bufs=2, space="PSUM") as ps_t,
            tc.tile_pool(name="ps_o", bufs=2, space="PSUM") as ps_o,
            tc.tile_pool(name="ps_c", bufs=2, space="PSUM") as ps_c,
        ):
            from concourse.masks import make_identity

            ident = consts.tile([128, 128], F16)
            make_identity(nc, ident)

            ctx_ps = {}
            ctxa = {}
            eqTs = {}  # (b, i) -> list of NCHUNK staged eqT tiles

            def emit_kv_block(b, i):
                n0 = i * LOAD
                k_sb = io.tile([128, LT, 64], F32, tag="k_sb", bufs=4)
                v32 = io.tile([128, LT, 64], F32, tag="v32", bufs=4)
                nc.sync.dma_start(out=k_sb, in_=blk(k, b, n0))
                nc.sync.dma_start(out=v32, in_=blk(v, b, n0))
                # ctx = (ek/ks)^T v == ek^T (v/ks): normalize v instead of ek,
                # fusing the softmax scale into the v f32->f16 cast.
                ek = work.tile([128, LT, 64], F16, tag="ek")
                nc.scalar.activation(ek, k_sb, EXP)
                ks = work.tile([128, LT, 1], F32, tag="ks")
                nc.vector.reduce_sum(out=ks, in_=ek, axis=mybir.AxisListType.X)
                ksr = work.tile([128, LT, 1], F32, tag="ksr")
                nc.vector.reciprocal(ksr, ks)
                v_sb = work.tile([128, LT, 64], F16, tag="v_sb", bufs=4)
                nc.vector.tensor_mul(v_sb, v32, ksr[:].to_broadcast((128, LT, 64)))
                for t in range(LT):
                    nc.tensor.matmul(
                        ctx_ps[b],
                        ek[:, t, :],
                        v_sb[:, t, :],
                        start=(i == 0 and t == 0),
                        stop=(i == NBLK - 1 and t == LT - 1),
                    )

            def emit_ctx_epilogue(b):
                ca = ctxp.tile([128, 130], F16, tag="ctxa")
                nc.vector.memset(ca, 0.0)
                nc.vector.tensor_copy(ca[0:64, 0:64], ctx_ps[b])
                nc.vector.memset(ca[0:64, 64:65], 1.0)
                nc.scalar.dma_start(out=ca[64:128, 65:130], in_=ca[0:64, 0:65])
                return ca

            pending_store = []

            def flush_stores():
                while pending_store:
                    dst, src = pending_store.pop(0)
                    nc.scalar.dma_start(out=dst, in_=src)

            def emit_qA_block(b, i, via_sync=False):
                """Load + transpose + exp: no ctx dependency.

                via_sync: load f32 on the sync (kv) queue instead of SWDGE --
                used for the last batch so its q loads queue AFTER all kv
                loads, keeping the DMA busy through the qB drain; the f32->f16
                cast runs on the otherwise-idle gpsimd.
                """
                if via_sync:
                    q32 = io.tile([128, LT, 64], F32, tag="q32", bufs=2)
                    nc.sync.dma_start(out=q32, in_=blk(q, b, i * LOAD))
                    q_sb = io.tile([128, LT, 64], F16, tag="q_sb", bufs=6, name="q_sb")
                    nc.gpsimd.tensor_copy(q_sb, q32)
                else:
                    q_sb = io.tile([128, LT, 64], F16, tag="q_sb", bufs=6, name="q_sb")
                    # SWDGE DMA casts f32 -> f16 in flight
                    nc.gpsimd.dma_start(out=q_sb, in_=blk(q, b, i * LOAD))
                tiles = []
                for c in range(NCHUNK):
                    # padded to 2 KB so each buf owns a full PSUM bank
                    tp_ps = ps_t.tile([128, 4, 256], F16, tag="tp_ps")
                    for u in range(4):
                        s0 = 8 * c + 2 * u
                        nc.tensor.transpose(
                            tp_ps[:, u, 0:128],
                            q_sb[:, s0 : s0 + 2, :].rearrange("p t d -> p (t d)"),
                            ident,
                        )
                    eqT = work.tile([128, 4, 128], F16, tag="eqT", bufs=20)
                    nc.scalar.activation(eqT, tp_ps[:, :, 0:128], EXP)
                    tiles.append(eqT)
                eqTs[(b, i)] = tiles

            def emit_qB_block(b, i, ca, split_store=False):
                """Matmul vs ctxa + normalize + store: needs ctx."""
                n0 = i * LOAD
                flush_stores()
                out_sb = io.tile([128, LT, 64], F16, tag="out_sb", bufs=3)
                tiles = eqTs.pop((b, i))
                for c in range(NCHUNK):
                    eqT = tiles[c]
                    # one 2-bank PSUM tile per chunk: slot stride 256 f32 (1 KB)
                    # keeps each 130-col matmul write inside a single bank
                    o_ps = ps_o.tile([128, 4, 256], F32, tag="o_ps")
                    for s in range(4):
                        nc.tensor.matmul(
                            o_ps[:, s, 0:130],
                            eqT[:, s, :],
                            ca,
                            start=True,
                            stop=True,
                        )
                    opb = o_ps[:]
                    pdim = opb.ap[0]
                    sstep = opb.ap[1][0]  # slot stride (256)
                    cstep = opb.ap[2][0]  # col stride (1)
                    r_sb = work.tile([128, 4, 2, 1], F32, tag="r_sb")
                    rs_ap = bass.AP(
                        tensor=opb.tensor,
                        offset=opb.offset + 64 * cstep,
                        ap=[pdim, [sstep, 4], [65 * cstep, 2], [cstep, 1]],
                    )
                    nc.vector.reciprocal(r_sb, rs_ap)
                    vals_ap = bass.AP(
                        tensor=opb.tensor,
                        offset=opb.offset,
                        ap=[pdim, [sstep, 4], [65 * cstep, 2], [cstep, 64]],
                    )
                    out_view = out_sb[:, 8 * c : 8 * c + 8, :].rearrange(
                        "p (s t) d -> p s t d", s=4
                    )
                    nc.vector.tensor_mul(
                        out_view,
                        vals_ap,
                        r_sb[:].to_broadcast((128, 4, 2, 64)),
                    )
                    if split_store:
                        nc.scalar.dma_start(
                            out=blk(o, b, n0)[:, 8 * c : 8 * c + 8, :],
                            in_=out_sb[:, 8 * c : 8 * c + 8, :],
                        )
                if not split_store:
                    pending_store.append((blk(o, b, n0), out_sb[:]))

            # ---- schedule ----
            # group 0:    kv(0,i) + qA(0,i)
            # group b:    kv(b,i) + qA(b,i) + qB(b-1,i)   [qA(3) deferred]
            # group last: qA(3,i) + qB(3,i)
            ctx_ps[0] = ps_c.tile([64, 64], F32, tag="ctx_ps", name="ctx_ps")
            for i in range(NBLK):
                emit_kv_block(0, i)
                emit_qA_block(0, i)
            ctxa[0] = emit_ctx_epilogue(0)
            for b in range(1, BPC):
                ctx_ps[b] = ps_c.tile([64, 64], F32, tag="ctx_ps", name="ctx_ps")
                for i in range(NBLK):
                    emit_kv_block(b, i)
                    if b < BPC - 1:
                        emit_qA_block(b, i)
                    emit_qB_block(b - 1, i, ctxa[b - 1])
                ctxa[b] = emit_ctx_epilogue(b)
            for i in range(NBLK):
                emit_qA_block(BPC - 1, i, via_sync=True)
                emit_qB_block(
                    BPC - 1, i, ctxa[BPC - 1], split_store=(i >= NBLK - 2)
                )
            flush_stores()

    nc.compile()
    return nc


_NC_CACHE = None


def kernel(q: np.ndarray, k: np.ndarray, v: np.ndarray) -> np.ndarray:
    global _NC_CACHE
    if _NC_CACHE is None:
        _NC_CACHE = build_bass()
    nc = _NC_CACHE
    q = np.ascontiguousarray(np.asarray(q), dtype=np.float32)
    k = np.ascontiguousarray(np.asarray(k), dtype=np.float32)
    v = np.ascontiguousarray(np.asarray(v), dtype=np.float32)
    in_maps = [
        {
            "q": q[i * BPC : (i + 1) * BPC],
            "k": k[i * BPC : (i + 1) * BPC],
            "v": v[i * BPC : (i + 1) * BPC],
        }
        for i in range(NCORES)
    ]
    res = run_bass_kernel_spmd(nc, in_maps, core_ids=list(range(NCORES)))
    out = np.concatenate([res.results[i]["o"] for i in range(NCORES)], axis=0)
    return out.astype(np.float32)


# revision 29
# speedup vs baseline: 1.1813x; 1.1813x over previous
"""EfficientAttention (linear attention) Trainium2 Bass kernel.

Computes, per batch b:
    q_n = softmax(q[b], axis=-1)        # over feature dim D=64
    k_n = softmax(k[b], axis=-1)
    ctx = k_n^T @ v[b]                  # [D, D]
    out[b] = q_n @ ctx                  # [N, D]

Sharding: batch dim (32) split across 8 cores, 4 batches per core.

Design notes (per core; memory-bound: 48 MB reads + 8 MB f16 writes at
~358 GB/s HBM-per-NC is the floor):
- k, v loaded f32 via HWDGE (sync queue); q via SWDGE (gpsimd queue)
  with in-flight f32->f16 cast, so q has its own DMA queue and arrives
  early while raw-q transposes become 1-pass fp16 with fast WL.
- kv pass: ctx = (ek/ks)^T v == ek^T (v/ks) -- ACT writes exp(k)
  directly as f16, DVE row-sums + reciprocal, and the softmax scale is
  fused into the v f32->f16 cast (one DVE broadcast multiply). No
  gpsimd in the chain.
- Output stored f16 (halves store traffic; host upcasts; 2e-2 budget).
- Q pass is split in two stages to keep every in-order engine stream
  consistent with DMA arrival order:
    qA (no ctx needed): load q, PE-transpose f16 pairs -> PSUM f16
       (full-bank tiles), ACT exp evict -> staged eqT tiles in SBUF.
    qB (needs ctxa): one matmul per tile pair into a single 2-bank
       PSUM tile (slot stride 1 KB so no 130-col write crosses a
       bank), ONE DVE reciprocal + ONE DVE normalize mul per chunk,
       deferred store trigger.
- ctx epilogue: block-diagonal stacked ctxa [128, 130] fp16 with ones
  columns so the q matmul also produces exp(q) row sums (cols 64/129).
- Schedule: slot s of batch b emits kv(b), qA(b), qB(b-1). qA(3) is
  reserved for the final group so its 4 MB of q loads (plus batch 3
  stores) keep the DMA busy while qB(3) drains; the exposed tail is
  one chunk chain, not a batch.
- Store triggers (ACT/HWDGE) are deferred one block so the ACT stream
  never stalls waiting on the DVE normalize.
"""

import numpy as np

import concourse.bass as bass
import concourse.mybir as mybir
import concourse.tile as tile
from concourse import bacc
from concourse.bass_utils import run_bass_kernel_spmd

B, N, D = 32, 16384, 64
NCORES = 8
BPC = B // NCORES  # batches per core
LOAD = 4096  # rows per DMA (1 MB fp32)
LT = LOAD // 128  # row-tile slots per load (32)
NBLK = N // LOAD  # load blocks per batch (4)
NCHUNK = LT // 8  # 1024-row compute chunks per block (4)
F32 = mybir.dt.float32
F16 = mybir.dt.float16
EXP = mybir.ActivationFunctionType.Exp

# last batch's q loads ride the sync queue (f32, after all kv loads) with a
# gpsimd cast, so the DMA stays busy through the final qB drain
QA3_VIA_SYNC = True


def build_bass():
    nc = bacc.Bacc("TRN2", target_bir_lowering=False, debug=False)
    q = nc.dram_tensor("q", [BPC, N, D], F32, kind="ExternalInput").ap()
    k = nc.dram_tensor("k", [BPC, N, D], F32, kind="ExternalInput").ap()
    v = nc.dram_tensor("v", [BPC, N, D], F32, kind="ExternalInput").ap()
    o = nc.dram_tensor("o", [BPC, N, D], F16, kind="ExternalOutput").ap()

    def blk(t, b, n0):
        return t[b, n0 : n0 + LOAD, :].rearrange("(p t) d -> p t d", p=128)

    with tile.TileContext(nc) as tc:
        with (
            tc.tile_pool(name="consts", bufs=1) as consts,
            tc.tile_pool(name="io", bufs=2) as io,
            tc.tile_pool(name="work", bufs=3) as work,
            tc.tile_pool(name="ctxp", bufs=3) as ctxp,
            tc.tile_pool(name="ps_t", bufs=2, space="PSUM") as ps_t,
            tc.tile_pool(name="ps_o", bufs=2, space="PSUM") as ps_o,
            tc.tile_pool(name="ps_c", bufs=2, space="PSUM") as ps_c,
        ):
            from concourse.masks import make_identity

            ident = consts.tile([128, 128], F16)
            make_identity(nc, ident)

            ctx_ps = {}
            ctxa = {}
            eqTs = {}  # (b, i) -> list of NCHUNK staged eqT tiles

            def emit_kv_block(b, i):
                n0 = i * LOAD
                k_sb = io.tile([128, LT, 64], F32, tag="k_sb", bufs=4)
                v_sb = io.tile([128, LT, 64], F16, tag="v_sb", bufs=4)
                nc.sync.dma_start(out=k_sb, in_=blk(k, b, n0))
                # SWDGE DMA casts v f32 -> f16 in flight
                nc.gpsimd.dma_start(out=v_sb, in_=blk(v, b, n0))
                ek = work.tile([128, LT, 64], F16, tag="ek")
                nc.scalar.activation(ek, k_sb, EXP)
                # DVE does only the small row-sum + reciprocal; the softmax
                # scale lands on gpsimd so DVE (which also carries the qB
                # normalize) stays off the ctx critical path
                ks = work.tile([128, LT, 1], F32, tag="ks")
                nc.vector.reduce_sum(out=ks, in_=ek, axis=mybir.AxisListType.X)
                ksr = work.tile([128, LT, 1], F32, tag="ksr")
                nc.vector.reciprocal(ksr, ks)
                ekn = work.tile([128, LT, 64], F16, tag="ekn", bufs=4)
                nc.gpsimd.tensor_mul(ekn, ek, ksr[:].to_broadcast((128, LT, 64)))
                for t in range(LT):
                    nc.tensor.matmul(
                        ctx_ps[b],
                        ekn[:, t, :],
                        v_sb[:, t, :],
                        start=(i == 0 and t == 0),
                        stop=(i == NBLK - 1 and t == LT - 1),
                    )

            def emit_ctx_epilogue(b):
                ca = ctxp.tile([128, 130], F16, tag="ctxa")
                nc.vector.memset(ca, 0.0)
                nc.vector.tensor_copy(ca[0:64, 0:64], ctx_ps[b])
                nc.vector.memset(ca[0:64, 64:65], 1.0)
                nc.scalar.dma_start(out=ca[64:128, 65:130], in_=ca[0:64, 0:65])
                return ca

            pending_store = []

            def flush_stores():
                while pending_store:
                    dst, src = pending_store.pop(0)
                    nc.scalar.dma_start(out=dst, in_=src)

            def emit_qA_block(b, i, via_sync=False):
                """Load + transpose + exp: no ctx dependency.

                via_sync: load f32 on the sync (kv) queue instead of SWDGE --
                used for the last batch so its q loads queue AFTER all kv
                loads, keeping the DMA busy through the qB drain; the f32->f16
                cast runs on the otherwise-idle gpsimd.
                """
                if via_sync:
                    q32 = io.tile([128, LT, 64], F32, tag="q32", bufs=2)
                    nc.sync.dma_start(out=q32, in_=blk(q, b, i * LOAD))
                    q_sb = io.tile([128, LT, 64], F16, tag="q_sb", bufs=6, name="q_sb")
                    nc.gpsimd.tensor_copy(q_sb, q32)
                else:
                    q_sb = io.tile([128, LT, 64], F16, tag="q_sb", bufs=6, name="q_sb")
                    # SWDGE DMA casts f32 -> f16 in flight
                    nc.gpsimd.dma_start(out=q_sb, in_=blk(q, b, i * LOAD))
                tiles = []
                for c in range(NCHUNK):
                    # padded to 2 KB so each buf owns a full PSUM bank
                    tp_ps = ps_t.tile([128, 4, 256], F16, tag="tp_ps")
                    for u in range(4):
                        s0 = 8 * c + 2 * u
                        nc.tensor.transpose(
                            tp_ps[:, u, 0:128],
                            q_sb[:, s0 : s0 + 2, :].rearrange("p t d -> p (t d)"),
                            ident,
                        )
                    eqT = work.tile([128, 4, 128], F16, tag="eqT", bufs=20)
                    nc.scalar.activation(eqT, tp_ps[:, :, 0:128], EXP)
                    tiles.append(eqT)
                eqTs[(b, i)] = tiles

            def emit_qB_block(b, i, ca, split_store=False):
                """Matmul vs ctxa + normalize + store: needs ctx."""
                n0 = i * LOAD
                flush_stores()
                out_sb = io.tile([128, LT, 64], F16, tag="out_sb", bufs=3)
                tiles = eqTs.pop((b, i))
                for c in range(NCHUNK):
                    eqT = tiles[c]
                    # one 2-bank PSUM tile per chunk: slot stride 256 f32 (1 KB)
                    # keeps each 130-col matmul write inside a single bank
                    o_ps = ps_o.tile([128, 4, 256], F32, tag="o_ps")
                    for s in range(4):
                        nc.tensor.matmul(
                            o_ps[:, s, 0:130],
                            eqT[:, s, :],
                            ca,
                            start=True,
                            stop=True,
                        )
                    opb = o_ps[:]
                    pdim = opb.ap[0]
                    sstep = opb.ap[1][0]  # slot stride (256)
                    cstep = opb.ap[2][0]  # col stride (1)
                    r_sb = work.tile([128, 4, 2, 1], F32, tag="r_sb")
                    rs_ap = bass.AP(
                        tensor=opb.tensor,
                        offset=opb.offset + 64 * cstep,
                        ap=[pdim, [sstep, 4], [65 * cstep, 2], [cstep, 1]],
                    )
                    nc.vector.reciprocal(r_sb, rs_ap)
                    vals_ap = bass.AP(
                        tensor=opb.tensor,
                        offset=opb.offset,
                        ap=[pdim, [sstep, 4], [65 * cstep, 2], [cstep, 64]],
                    )
                    out_view = out_sb[:, 8 * c : 8 * c + 8, :].rearrange(
                        "p (s t) d -> p s t d", s=4
                    )
                    nc.vector.tensor_mul(
                        out_view,
                        vals_ap,
                        r_sb[:].to_broadcast((128, 4, 2, 64)),
                    )
                    if split_store:
                        nc.scalar.dma_start(
                            out=blk(o, b, n0)[:, 8 * c : 8 * c + 8, :],
                            in_=out_sb[:, 8 * c : 8 * c + 8, :],
                        )
                if not split_store:
                    pending_store.append((blk(o, b, n0), out_sb[:]))

            # ---- schedule: kv two batches ahead of qB ----
            # kv(0) | kv(1)+qA(0) | kv(2)+qA(1)+qB(0) | kv(3)+qA(2)+qB(1)
            #       | qA(3)+qB(2) | qB(3)
            ctx_ps[0] = ps_c.tile([64, 64], F32, tag="ctx_ps", name="ctx_ps")
            for i in range(NBLK):
                emit_kv_block(0, i)
            ctxa[0] = emit_ctx_epilogue(0)
            ctx_ps[1] = ps_c.tile([64, 64], F32, tag="ctx_ps", name="ctx_ps")
            for i in range(NBLK):
                emit_kv_block(1, i)
                emit_qA_block(0, i)
            ctxa[1] = emit_ctx_epilogue(1)
            for b in range(1, BPC - 1):
                ctx_ps[b + 1] = ps_c.tile([64, 64], F32, tag="ctx_ps", name="ctx_ps")
                for i in range(NBLK):
                    emit_kv_block(b + 1, i)
                    emit_qA_block(b, i)
                    emit_qB_block(b - 1, i, ctxa[b - 1])
                ctxa[b + 1] = emit_ctx_epilogue(b + 1)
            for i in range(NBLK):
                emit_qA_block(BPC - 1, i)
                emit_qB_block(BPC - 2, i, ctxa[BPC - 2])
            for i in range(NBLK):
                emit_qB_block(
                    BPC - 1, i, ctxa[BPC - 1], split_store=(i >= NBLK - 2)
                )
            flush_stores()

    nc.compile()
    return nc


_NC_CACHE = None


def kernel(q: np.ndarray, k: np.ndarray, v: np.ndarray) -> np.ndarray:
    global _NC_CACHE
    if _NC_CACHE is None:
        _NC_CACHE = build_bass()
    nc = _NC_CACHE
    q = np.ascontiguousarray(np.asarray(q), dtype=np.float32)
    k = np.ascontiguousarray(np.asarray(k), dtype=np.float32)
    v = np.ascontiguousarray(np.asarray(v), dtype=np.float32)
    in_maps = [
        {
            "q": q[i * BPC : (i + 1) * BPC],
            "k": k[i * BPC : (i + 1) * BPC],
            "v": v[i * BPC : (i + 1) * BPC],
        }
        for i in range(NCORES)
    ]
    res = run_bass_kernel_spmd(nc, in_maps, core_ids=list(range(NCORES)))
    out = np.concatenate([res.results[i]["o"] for i in range(NCORES)], axis=0)
    return out.astype(np.float32)
